# revision 1
# baseline (speedup 1.0000x reference)
"""Trainium2 Bass kernel for nn_Attention_7078106104284.

Self-attention block (SAGAN-style) over x[8, 256, 64, 64]:
  q = wq@x+bq [32,n], k = wk@x+bk [32,n], v = wv@x+bv [256,n], n = 4096
  attn = softmax(q^T k, axis=m);  y = x + gamma * (v @ attn^T)

Sharding: data-parallel over batch — one batch element per NeuronCore (8 cores).

All matmuls are bf16 (fp32/f32r matmuls measure ~1us each on TRN2 — no fast
weight load, no back-to-back pipelining — while bf16 streams at rate).
Precision-critical products use a hi/lo split: a ~ a_hi + a_lo with both bf16
captures ~16 mantissa bits; a*b ~ a_hi*b_hi + a_hi*b_lo + a_lo*b_hi (the
dropped lo*lo term is ~1e-5 relative), accumulated exactly in fp32 PSUM.

Per-core algorithm:
  - q/k projections: 3-term split (weights pre-split on host, x pre-split on
    host into x_hi/x_lo bf16), 6 accumulating matmuls per 512-wide n-tile per
    head; q_hi/k_hi written by ACT (bias fused), q_lo/k_lo by DVE.
  - Logit stacks: q_stack = [q_hi; q_hi; q_lo], k_stack = [k_lo; k_hi; k_hi]
    along partitions, so each 128-m-chunk of transposed logits
    Lt[m, n] = sum_o k[o,m] q[o,n] is ONE K=96 bf16 matmul of N=512.
  - exp fused with PSUM->SBUF evacuation on ACT over [128, 4*512] spans,
    bf16 out. Softmax max-subtraction skipped: |logit| < 50 << 88.
  - vT[m, c'] via plain bf16 (x_hi only — v's error is gamma-damped), with a
    ones column at c' = 256 (tiny K=1 matmul) so the softmax denominator Z
    rides the AV product for free.
  - AV transposed: uT[n, c'] = sum_m e[m, n] vT[m, c'] per 128-wide n-sub.
  - Normalize on DVE: uTn = uT * (gamma/Z[n]) per-partition, bf16; PE
    transposes uTn back to [c, n]; epilogue y = trans + x + gamma*bv
    (v's bias folds into the residual because sum_m attn = 1).
"""

import sys

sys.path.insert(0, "/opt/trn_rl_repo")

import numpy as np
from contextlib import ExitStack

import concourse.bass as bass
import concourse.bacc as bacc
import concourse.tile as tile
import concourse.mybir as mybir
from concourse.masks import make_identity
from concourse.bass_utils import run_bass_kernel_spmd

dt = mybir.dt
AF = mybir.ActivationFunctionType

B = 8
C = 256
C8 = 32
N = 4096          # h*w spatial positions
NG = 512          # n-group width (one PSUM bank of fp32)
G = N // NG       # 8 n-groups
MC = N // 128     # 32 m-chunks
EW = 4            # m-chunks per exp batch (PSUM banks per plt tile)
RND = MC // EW    # rounds per group
CP = C + 1        # AV output channels incl. the Z ones-column


def build_program(reps=1, ablate=()):
    nc = bacc.Bacc("TRN2", target_bir_lowering=False)
    f32 = dt.float32
    bf16 = dt.bfloat16
    x_d = nc.declare_dram_parameter("x", [C, N], f32, isOutput=False)
    xhi_d = nc.declare_dram_parameter("x_hi", [C, N], bf16, isOutput=False)
    xlo_d = nc.declare_dram_parameter("x_lo", [C, N], bf16, isOutput=False)
    # pre-split weights: [hi, lo] stacked on axis 0
    wq_d = nc.declare_dram_parameter("wqT_hl", [2 * C, C8], bf16, isOutput=False)
    wk_d = nc.declare_dram_parameter("wkT_hl", [2 * C, C8], bf16, isOutput=False)
    wv_d = nc.declare_dram_parameter("wvT_h", [C, C], bf16, isOutput=False)
    bq_d = nc.declare_dram_parameter("bq", [C8, 1], f32, isOutput=False)
    bk_d = nc.declare_dram_parameter("bk", [C8, 1], f32, isOutput=False)
    bv_d = nc.declare_dram_parameter("bv", [128, 2], f32, isOutput=False)
    gamma_d = nc.declare_dram_parameter("gamma", [1, 1], f32, isOutput=False)
    y_d = nc.declare_dram_parameter("y", [C, N], f32, isOutput=True)

    with tile.TileContext(nc) as tc, ExitStack() as ctx:
        sing = ctx.enter_context(tc.tile_pool(name="sing", bufs=1))
        epool = ctx.enter_context(tc.tile_pool(name="epool", bufs=RND + 2))
        upool = ctx.enter_context(tc.tile_pool(name="upool", bufs=6))
        ypool = ctx.enter_context(tc.tile_pool(name="ypool", bufs=3))
        scal = ctx.enter_context(tc.tile_pool(name="scal", bufs=4))
        lopool = ctx.enter_context(tc.tile_pool(name="lopool", bufs=3))

        lt_ps = ctx.enter_context(tc.tile_pool(name="lt_ps", bufs=1, space="PSUM"))
        u_ps = ctx.enter_context(tc.tile_pool(name="u_ps", bufs=1, space="PSUM"))

        for _rep in range(reps):
            # ---- static inputs ----
            x_sb = sing.tile([128, 2, N], f32)           # residual term
            nc.sync.dma_start(out=x_sb, in_=x_d[:].rearrange("(cc p) m -> p cc m", p=128))
            xhi_sb = sing.tile([128, 2, N], bf16)
            nc.sync.dma_start(out=xhi_sb, in_=xhi_d[:].rearrange("(cc p) m -> p cc m", p=128))
            xlo_sb = sing.tile([128, 2, N], bf16)
            nc.sync.dma_start(out=xlo_sb, in_=xlo_d[:].rearrange("(cc p) m -> p cc m", p=128))
            wq_sb = sing.tile([128, 4, C8], bf16)        # slots (hl, cc): 0=h0,1=h1,2=l0,3=l1
            nc.sync.dma_start(out=wq_sb, in_=wq_d[:].rearrange("(hl cc p) o -> p (hl cc) o", p=128, hl=2))
            wk_sb = sing.tile([128, 4, C8], bf16)
            nc.sync.dma_start(out=wk_sb, in_=wk_d[:].rearrange("(hl cc p) o -> p (hl cc) o", p=128, hl=2))
            wv_sb = sing.tile([128, 2, C], bf16)
            nc.sync.dma_start(out=wv_sb, in_=wv_d[:].rearrange("(cc p) c -> p cc c", p=128))
            bq_sb = sing.tile([C8, 1], f32)
            nc.sync.dma_start(out=bq_sb, in_=bq_d[:])
            bk_sb = sing.tile([C8, 1], f32)
            nc.sync.dma_start(out=bk_sb, in_=bk_d[:])
            bv_sb = sing.tile([128, 2], f32)
            nc.sync.dma_start(out=bv_sb, in_=bv_d[:])
            g128 = sing.tile([128, 1], f32)
            nc.sync.dma_start(
                out=g128,
                in_=bass.AP(tensor=gamma_d, offset=0, ap=[[0, 128], [1, 1]]),
            )

            ones_f32 = sing.tile([128, 1], f32)
            nc.vector.memset(ones_f32, 1.0)
            one_b = sing.tile([1, 1], bf16)              # K=1 ones-column writer
            nc.scalar.activation(one_b, ones_f32[0:1, :], AF.Copy)
            one_row_b = sing.tile([1, 128], bf16)
            nc.scalar.activation(
                one_row_b, bass.AP(tensor=ones_f32.tensor, offset=ones_f32.offset,
                                   ap=[[1, 1], [0, 128]]), AF.Copy)
            ident = sing.tile([128, 128], bf16)          # transpose identity
            make_identity(nc, ident)

            # gbv[c] = gamma * bv[c]  (per-partition adder for the epilogue)
            gbv = sing.tile([128, 2], f32)
            nc.vector.tensor_scalar_mul(gbv, bv_sb, g128)

            # ---- q/k projections with hi/lo split ----
            # logit strips: s0 = (q_hi, k_lo), s1 = (q_hi, k_hi), s2 = (q_lo, k_hi)
            q_stack = sing.tile([3 * C8, N], bf16)
            k_stack = sing.tile([3 * C8, N], bf16)
            qhi_t = sing.tile([C8, N], bf16)
            khi_t = sing.tile([C8, N], bf16)
            qlo_t = sing.tile([C8, N], bf16)
            klo_t = sing.tile([C8, N], bf16)
            # (w slot, x tile): hi*x_hi (cc0, cc1), hi*x_lo, lo*x_hi
            terms = [(0, xhi_sb), (1, xhi_sb), (0, xlo_sb), (1, xlo_sb),
                     (2, xhi_sb), (3, xhi_sb)]
            for s in range(G):
                sl = slice(s * NG, (s + 1) * NG)
                pq = u_ps.tile([C8, NG], f32, tag="u0", name="pq")
                pk = u_ps.tile([C8, NG], f32, tag="u1", name="pk")
                for i, (ws, xt) in enumerate(terms):
                    nc.tensor.matmul(pq, wq_sb[:, ws, :], xt[:, ws % 2, sl],
                                     start=(i == 0), stop=(i == len(terms) - 1))
                for i, (ws, xt) in enumerate(terms):
                    nc.tensor.matmul(pk, wk_sb[:, ws, :], xt[:, ws % 2, sl],
                                     start=(i == 0), stop=(i == len(terms) - 1))
                # hi = bf16(proj + bias) on ACT; lo = (proj + bias) - hi on DVE
                nc.scalar.activation(qhi_t[:, sl], pq, AF.Identity, bias=bq_sb)
                nc.scalar.activation(khi_t[:, sl], pk, AF.Identity, bias=bk_sb)
                qf = lopool.tile([C8, NG], f32, tag="qf", name="qf")
                nc.vector.tensor_scalar_add(qf, pq, bq_sb)
                nc.vector.tensor_sub(qlo_t[:, sl], qf, qhi_t[:, sl])
                kf = lopool.tile([C8, NG], f32, tag="kf", name="kf")
                nc.vector.tensor_scalar_add(kf, pk, bk_sb)
                nc.vector.tensor_sub(klo_t[:, sl], kf, khi_t[:, sl])
            # fill stack strips (SBUF->SBUF DMA can shift partitions)
            nc.sync.dma_start(out=q_stack[0:C8, :], in_=qhi_t)
            nc.sync.dma_start(out=q_stack[C8:2 * C8, :], in_=qhi_t)
            nc.sync.dma_start(out=q_stack[2 * C8:3 * C8, :], in_=qlo_t)
            nc.sync.dma_start(out=k_stack[0:C8, :], in_=klo_t)
            nc.sync.dma_start(out=k_stack[C8:2 * C8, :], in_=khi_t)
            nc.sync.dma_start(out=k_stack[2 * C8:3 * C8, :], in_=khi_t)

            # ---- vT[m, c'] in bf16 with ones column at c' = 256 ----
            vt_sb = sing.tile([128, MC, CP], bf16)
            for mc in range(MC):
                msl = slice(mc * 128, (mc + 1) * 128)
                pv = u_ps.tile([128, CP], f32, tag="u2", name="pv")
                for cc in range(2):
                    nc.tensor.matmul(pv[:, 0:C], xhi_sb[:, cc, msl], wv_sb[:, cc, :],
                                     start=(cc == 0), stop=(cc == 1))
                nc.tensor.matmul(pv[:, C:CP], one_row_b, one_b,
                                 start=True, stop=True)
                nc.scalar.activation(vt_sb[:, mc, :], pv, AF.Copy)

            # ---- attention, software-pipelined over n-groups ----
            e_tiles = {}
            u_tiles = {}

            def issue_lt_exp(g, j):
                # round j: logits for m-chunks EW*j..EW*j+EW-1, one K=96 bf16
                # matmul each into its own PSUM bank; one fused exp over all.
                sl = slice(g * NG, (g + 1) * NG)
                plt = lt_ps.tile([128, EW, NG], f32, tag="plt", name="plt")
                for rg in range(EW if "lt" not in ablate else 1):
                    mc = EW * j + rg
                    msl = slice(mc * 128, (mc + 1) * 128)
                    nc.tensor.matmul(plt[:, rg, :], k_stack[:, msl], q_stack[:, sl],
                                     start=True, stop=True)
                e_t = epool.tile([128, EW, NG], bf16, tag="e", name="e_t")
                fn = AF.Exp if "exp" not in ablate else AF.Copy
                h = EW // 2
                nc.scalar.activation(e_t[:, 0:h, :], plt[:, 0:h, :], fn)
                nc.scalar.activation(e_t[:, h:EW, :], plt[:, h:EW, :], fn)
                e_tiles[(g, j)] = e_t

            def issue_av(g, j):
                uts = u_tiles[g]
                e_t = e_tiles.pop((g, j))
                if "av" in ablate:
                    if j == 0:
                        for sub in range(4):
                            nc.tensor.matmul(uts[sub],
                                             e_t[:, 0, sub * 128:(sub + 1) * 128],
                                             vt_sb[:, 0, :], start=True, stop=True)
                    return
                for rg in range(EW):
                    mc = EW * j + rg
                    first = (j == 0 and rg == 0)
                    last = (j == RND - 1 and rg == EW - 1)
                    for sub in range(4):
                        nc.tensor.matmul(uts[sub],
                                         e_t[:, rg, sub * 128:(sub + 1) * 128],
                                         vt_sb[:, mc, :],
                                         start=first, stop=last)

            def issue_epilogue(g):
                uts = u_tiles.pop(g)
                # normalize per n-sub-block into bf16 hi+lo pair, transpose
                # both back to [c, n] (bf16 transposes stay on the fast PE
                # path); the four transpose PSUM tiles reuse the freed
                # u0..u3 slots. y = t_hi + t_lo + x + gamma*bv.
                tph = [u_ps.tile([128, NG], dt.bfloat16, tag=f"u{cb}", name="tph")
                       for cb in range(2)]
                tpl = [u_ps.tile([128, NG], dt.bfloat16, tag=f"u{cb + 2}", name="tpl")
                       for cb in range(2)]
                for sub in range(4):
                    ut = uts[sub]
                    rinv = scal.tile([128, 1], f32, tag="rinv", name="rinv")
                    nc.vector.reciprocal(rinv, ut[:, C:CP])
                    gsc = scal.tile([128, 1], f32, tag="gsc", name="gsc")
                    nc.vector.tensor_scalar_mul(gsc, rinv, g128)
                    unf = upool.tile([128, C], f32, tag="unf", name="unf")
                    nc.vector.tensor_scalar_mul(unf, ut[:, 0:C], gsc)
                    un = upool.tile([128, C], dt.bfloat16, tag="un", name="un")
                    nc.vector.tensor_copy(un, unf)
                    unl = upool.tile([128, C], dt.bfloat16, tag="unl", name="unl")
                    nc.vector.tensor_sub(unl, unf, un)
                    for cb in range(2):
                        nc.tensor.transpose(
                            tph[cb][:, sub * 128:(sub + 1) * 128],
                            un[:, cb * 128:(cb + 1) * 128], ident)
                        nc.tensor.transpose(
                            tpl[cb][:, sub * 128:(sub + 1) * 128],
                            unl[:, cb * 128:(cb + 1) * 128], ident)
                sl = slice(g * NG, (g + 1) * NG)
                for cb in range(2):
                    y_t = ypool.tile([128, NG], f32, tag="y", name="y")
                    nc.vector.tensor_add(y_t, tph[cb], x_sb[:, cb, sl])
                    nc.vector.tensor_add(y_t, y_t, tpl[cb])
                    nc.vector.tensor_scalar_add(y_t, y_t, gbv[:, cb:cb + 1])
                    nc.sync.dma_start(
                        out=y_d[:].rearrange("(cc p) m -> p cc m", p=128)[:, cb, sl],
                        in_=y_t,
                    )

            for g in range(G + 1):
                if g < G:
                    u_tiles[g] = [u_ps.tile([128, CP], f32, tag=f"u{s}", name=f"u{s}")
                                  for s in range(4)]
                for j in range(RND):
                    if g < G:
                        issue_lt_exp(g, j)
                    if g >= 1:
                        issue_av(g - 1, j)
                if g >= 1:
                    issue_epilogue(g - 1)

    nc.compile()
    return nc


_nc_cache = None


def kernel(**inputs) -> np.ndarray:
    global _nc_cache
    import ml_dtypes
    bf = ml_dtypes.bfloat16
    x = np.asarray(inputs["x"], dtype=np.float32)
    wq = np.asarray(inputs["wq"], dtype=np.float32)
    bq = np.asarray(inputs["bq"], dtype=np.float32)
    wk = np.asarray(inputs["wk"], dtype=np.float32)
    bk = np.asarray(inputs["bk"], dtype=np.float32)
    wv = np.asarray(inputs["wv"], dtype=np.float32)
    bv = np.asarray(inputs["bv"], dtype=np.float32)
    gamma = np.asarray(inputs["gamma"], dtype=np.float32)

    if _nc_cache is None:
        _nc_cache = build_program()
    nc = _nc_cache

    xr = np.ascontiguousarray(x.reshape(B, C, N))
    x_hi = xr.astype(bf)
    x_lo = (xr - x_hi.astype(np.float32)).astype(bf)

    def hl(wT):   # [C, C8] fp32 -> [2C, C8] bf16 [hi; lo]
        hi = wT.astype(bf)
        lo = (wT - hi.astype(np.float32)).astype(bf)
        return np.ascontiguousarray(np.concatenate([hi, lo], axis=0))

    shared = {
        "wqT_hl": hl(wq.T),
        "wkT_hl": hl(wk.T),
        "wvT_h": np.ascontiguousarray(wv.T.astype(bf)),
        "bq": np.ascontiguousarray(bq.reshape(C8, 1)),
        "bk": np.ascontiguousarray(bk.reshape(C8, 1)),
        "bv": np.ascontiguousarray(bv.reshape(2, 128).T),
        "gamma": np.ascontiguousarray(gamma.reshape(1, 1)),
    }
    in_maps = [dict(shared, x=xr[i], x_hi=np.ascontiguousarray(x_hi[i]),
                    x_lo=np.ascontiguousarray(x_lo[i])) for i in range(B)]
    res = run_bass_kernel_spmd(nc, in_maps, core_ids=list(range(B)))
    y = np.stack([res.results[i]["y"] for i in range(B)], axis=0)
    return y.reshape(B, C, 64, 64).astype(np.float32)


if __name__ == "__main__":
    rng = np.random.default_rng(0)
    ins = {
        "x": rng.standard_normal((B, C, 64, 64), dtype=np.float32),
        "wq": rng.standard_normal((C8, C), dtype=np.float32) / 16,
        "bq": rng.standard_normal((C8,), dtype=np.float32) * 0.01,
        "wk": rng.standard_normal((C8, C), dtype=np.float32) / 16,
        "bk": rng.standard_normal((C8,), dtype=np.float32) * 0.01,
        "wv": rng.standard_normal((C, C), dtype=np.float32) / 16,
        "bv": rng.standard_normal((C,), dtype=np.float32) * 0.01,
        "gamma": rng.standard_normal((1,), dtype=np.float32) * 0.1,
    }
    out = kernel(**ins)
    print("kernel output", out.shape, out.dtype)

